# revision 4
# baseline (speedup 1.0000x reference)
"""Trainium2 Bass kernel for nn_BiPartiteAssignmentCriterion.

Reference computation:
    label_loss = mean over pre-EOS positions of  -log_softmax(logits)[b, t, matched[b, t]]
    eos_loss   = mean over EOS/pad positions of  -log_softmax(logits)[b, t, 0]
where `matched` comes from a per-sample Hungarian matching that the reference
itself performs on host CPU (detached from the device graph).

Split used here (data-parallel over batch, 8 NeuronCores):
  - Device: the memory-bound part -- read all 256*21*32000 fp32 logits
    (688 MB; 86 MB per core) and compute sum(exp(x)) per (b, t) row, i.e.
    the softmax denominator. Each core handles 672 rows, laid out as five
    [128, 32000] blocks plus one [32, 32000] block viewed as [128, 8000].
    exp + free-dim accumulation run on the ACT (scalar) engine, hidden
    behind the HBM DMA stream.
  - Host: Hungarian matching (identical algorithm to the reference), the
    5376-element gathers of matched/EOS logits, and the final two scalar
    means.  nll = log(sumexp) - logit[target].
"""

import numpy as np

B, T, V = 256, 21, 32000
EOS = 2
NCORES = 8
ROWS = B * T                 # 5376
R = ROWS // NCORES           # 672 rows per core
NFULL = R // 128             # 5 full 128-row blocks per core
REM = R - NFULL * 128        # 32 leftover rows
REMF = REM * V // 128        # leftover block viewed as [128, 8000]
CH = 8000                    # free-dim chunk per DMA / ACT instruction

_CACHE = {}


def _build_bass():
    import concourse.bacc as bacc
    import concourse.mybir as mybir
    import concourse.tile as tile

    fp32 = mybir.dt.float32
    # Bacc (not raw Bass): its finalize runs generate_event_semaphores(),
    # which splits multi-wait instructions into EventSemaphore pairs --
    # TRN2 codegen allows at most one embedded sync wait per instruction.
    nc = bacc.Bacc()
    x = nc.declare_dram_parameter("x", [R * V], fp32, isOutput=False)
    out = nc.declare_dram_parameter("sums", [128, NFULL + 1], fp32, isOutput=True)

    with tile.TileContext(nc) as tc:
        with (
            tc.tile_pool(name="xin", bufs=3) as pin,
            tc.tile_pool(name="expo", bufs=2) as pexp,
            tc.tile_pool(name="acc", bufs=2) as pacc,
            tc.tile_pool(name="res", bufs=1) as pres,
        ):
            res = pres.tile([128, NFULL + 1], fp32)
            for b in range(NFULL + 1):
                if b < NFULL:
                    xb = x[b * 128 * V:(b + 1) * 128 * V].rearrange(
                        "(p f) -> p f", f=V
                    )
                else:
                    xb = x[NFULL * 128 * V:].rearrange("(p f) -> p f", f=REMF)
                nch = xb.shape[1] // CH
                acc = pacc.tile([128, nch], fp32, tag="acc")
                for c in range(nch):
                    xt = pin.tile([128, CH], fp32)
                    # SWDGE (gpsimd): sync waits are sequencer instructions,
                    # not embedded in the DMA -- HWDGE direct-2D lowering only
                    # supports a single embedded wait and slot-reuse needs two.
                    nc.gpsimd.dma_start(out=xt[:], in_=xb[:, c * CH:(c + 1) * CH])
                    et = pexp.tile([128, CH], fp32)
                    nc.scalar.activation(
                        out=et[:],
                        in_=xt[:],
                        func=mybir.ActivationFunctionType.Exp,
                        accum_out=acc[:, c:c + 1],
                    )
                nc.vector.reduce_sum(
                    out=res[:, b:b + 1], in_=acc[:], axis=mybir.AxisListType.X
                )
            nc.sync.dma_start(out=out[:, :], in_=res[:])
    # Bacc defers register allocation and wait legalization to finalize();
    # run_bass_via_pjrt serializes the module as-is, so finalize here.
    nc.finalize()
    return nc


def _hungarian_max_py(cost):
    """Square n x n assignment, maximize total cost (same algorithm as the
    reference / scipy linear_sum_assignment with maximize=True)."""
    c = -np.asarray(cost, dtype=np.float64)
    n = c.shape[0]
    INF = 1e18
    u = np.zeros(n + 1)
    v = np.zeros(n + 1)
    p = np.zeros(n + 1, dtype=np.int64)
    way = np.zeros(n + 1, dtype=np.int64)
    for i in range(1, n + 1):
        p[0] = i
        j0 = 0
        minv = np.full(n + 1, INF)
        used = np.zeros(n + 1, dtype=bool)
        while True:
            used[j0] = True
            i0 = p[j0]
            delta = INF
            j1 = 0
            for j in range(1, n + 1):
                if not used[j]:
                    cur = c[i0 - 1, j - 1] - u[i0] - v[j]
                    if cur < minv[j]:
                        minv[j] = cur
                        way[j] = j0
                    if minv[j] < delta:
                        delta = minv[j]
                        j1 = j
            for j in range(n + 1):
                if used[j]:
                    u[p[j]] += delta
                    v[j] -= delta
                else:
                    minv[j] -= delta
            j0 = j1
            if p[j0] == 0:
                break
        while j0:
            j1 = way[j0]
            p[j0] = p[j1]
            j0 = j1
    ans = np.zeros(n, dtype=np.int64)
    for j in range(1, n + 1):
        if p[j] > 0:
            ans[p[j] - 1] = j - 1
    return ans


def _hungarian_max(cost):
    try:
        from scipy.optimize import linear_sum_assignment
    except ImportError:
        return _hungarian_max_py(cost)
    _, col = linear_sum_assignment(np.asarray(cost, dtype=np.float64), maximize=True)
    return col.astype(np.int64)


def _match_perms(lg, tg, eos_idx):
    perm = np.zeros((B, T), dtype=np.int64)
    for i in range(B):
        L = int(eos_idx[i])
        if L > 0:
            cost = lg[i, :L][:, tg[i, :L]]
            perm[i, :L] = _hungarian_max(cost)
    return perm


def _run_device(lg, trace=False, **trace_kwargs):
    """lg: [B, T, V] fp32 contiguous. Returns (sumexp [B, T] float64, results)."""
    from concourse.bass_utils import run_bass_kernel_spmd

    if "nc" not in _CACHE:
        _CACHE["nc"] = _build_bass()
    nc = _CACHE["nc"]

    flat = lg.reshape(-1)
    per_core = R * V
    in_maps = [{"x": flat[c * per_core:(c + 1) * per_core]} for c in range(NCORES)]
    out = run_bass_kernel_spmd(
        nc, in_maps, core_ids=list(range(NCORES)), trace=trace, **trace_kwargs
    )

    sumexp = np.empty((NCORES, R), dtype=np.float64)
    for c in range(NCORES):
        arr = out.results[c]["sums"].astype(np.float64)  # [128, NFULL+1]
        sumexp[c, : NFULL * 128] = arr[:, :NFULL].T.reshape(-1)
        sumexp[c, NFULL * 128:] = arr[:, NFULL].reshape(REM, 128 // REM).sum(axis=1)
    return sumexp.reshape(B, T), out


def _loss_from_sumexp(lg, tg, sumexp):
    eos_idx = np.argmax(tg == EOS, axis=1)
    perm = _match_perms(lg, tg, eos_idx)
    matched = np.take_along_axis(tg, perm, axis=1)

    pos = np.arange(T)[None, :]
    label_mask = pos < eos_idx[:, None]
    eos_mask = ~label_mask

    tok = np.where(label_mask, matched, 0)
    x_tok = np.take_along_axis(lg, tok[:, :, None], axis=2)[:, :, 0].astype(np.float64)
    x0 = lg[:, :, 0].astype(np.float64)
    lse = np.log(sumexp)

    label_loss = (lse - x_tok)[label_mask].sum() / label_mask.sum()
    eos_loss = (lse - x0)[eos_mask].sum() / eos_mask.sum()
    return np.array([label_loss, eos_loss], dtype=np.float32)


def kernel(logits, targets):
    lg = np.ascontiguousarray(np.asarray(logits, dtype=np.float32))
    tg = np.asarray(targets)
    assert lg.shape == (B, T, V), lg.shape
    sumexp, _ = _run_device(lg, trace=False)
    return _loss_from_sumexp(lg, tg, sumexp)


def run_with_timing(logits, targets, **trace_kwargs):
    """Like kernel(), but traces the device run. Returns (result, BassKernelResults)."""
    lg = np.ascontiguousarray(np.asarray(logits, dtype=np.float32))
    tg = np.asarray(targets)
    sumexp, out = _run_device(lg, trace=True, **trace_kwargs)
    return _loss_from_sumexp(lg, tg, sumexp), out
